# revision 25
# baseline (speedup 1.0000x reference)
"""SAGAN-style self-attention block on 8 Trainium2 NeuronCores.

Reference computation (per batch element b, data-parallel over B=8):
    theta = w_theta @ x                         [16, 4096]
    phi   = maxpool2x2(w_phi @ x)               [16, 1024]
    g     = maxpool2x2(w_g @ x)                 [64, 1024]
    scores= theta^T @ phi                       [4096, 1024]
    beta  = softmax_j(scores)
    o     = g @ beta^T                          [64, 4096]
    out   = gamma * (w_o @ o) + x               [128, 4096]

Device algorithm (one batch element per core):
  * theta is never materialized: scores^T = A^T @ x with A = w_theta^T @ phi,
    so the big matmul runs with K=128 instead of K=16.
  * scores^T is kept j-on-partitions; softmax runs without max subtraction
    (max |score| ~ 37 for this distribution, exp stays in fp32/bf16 range).
  * exp is split across the Activation engine (exact) and the Vector engine
    (Schraudolph bit-trick exp via tensor_scalar f32->int16 bitcast bf16,
    ~3% per-element error, calibrated for the HW's round-to-nearest).
  * o is accumulated TRANSPOSED: out[i, c] = sum_j E[j, i] * gaug[j, c] with
    i on all 128 partitions (PE cost is per output column, so [128, 65] out
    beats [65, 512] out by ~2x). gaug's column 64 is all-ones, which makes
    the softmax denominator land as a per-partition column for free.
  * normalization = reciprocal of the denominator column [128, 4] + a
    per-partition-scalar tensor_scalar multiply fused into the mandatory
    PSUM->SBUF evacuation (bf16 out).
  * o_norm^T tiles are transposed back to [c, i] with one batched XBAR
    transpose DMA per icg round (src [128, (64c x 8k)], 3D dest AP).
  * output conv (gamma folded into w_o on host) + residual add (reads the
    bf16 x copy) evacuates the conv PSUM via one DVE tensor_add per chunk.
"""

import numpy as np

import concourse.bass as bass
import concourse.bacc as bacc
import concourse.tile as tile
from concourse import mybir
from concourse.bass_utils import run_bass_kernel_spmd

F32 = mybir.dt.float32
BF16 = mybir.dt.bfloat16
I16 = mybir.dt.int16

C = 128          # channels
HW = 4096        # 64*64 spatial
HWP = 1024       # pooled spatial (32*32)
C8 = 16          # C // 8
C2 = 64          # C // 2
NCORES = 8

# Schraudolph fast-exp constants (bf16 via int16): E = bitcast(int16(A*s + B)).
# Bias calibrated for round-to-nearest (what HW does; sim truncates, which
# only matters for sim-side numerics, not timing).
SCH_A = float(2**7 / np.log(2.0))
SCH_B = float(127 * 2**7) - 5.5

# x tile layout: sizes (in cols) of the bf16 x tiles; first two smaller so
# the pre-phase pipeline starts earlier.
XT = (256, 256, 512, 512, 512, 1024, 1024)
XO = (0, 256, 512, 1024, 1536, 2048, 3072)

# (icg, jc) pairs whose exp runs on the Vector engine via fast-exp.
DVE_EXP = {(0, 4), (0, 5), (1, 3), (1, 6), (2, 3), (2, 6), (3, 2)}

LAST_RESULTS = None


def _emit(nc: bass.Bass, tc: tile.TileContext, x_d, wgp_d, wth_d, wog_d, out_d, pfx=""):
    import contextlib

    with contextlib.ExitStack() as ctx:
        singles = ctx.enter_context(tc.tile_pool(name=pfx + "singles", bufs=1))

        # dummy exp so the ACT function table loads at kernel start
        dummy = singles.tile([1, 1], F32, tag="dummy")
        nc.vector.memset(dummy, 0.0)
        nc.scalar.activation(out=dummy, in_=dummy, func=mybir.ActivationFunctionType.Exp)

        # ---- constants / weights (triggered from ACT: idle during prologue) --
        wgp_sb = singles.tile([C, C2 + C8], BF16, tag="wgp")     # [128, 80]
        nc.sync.dma_start(out=wgp_sb, in_=wgp_d)
        wth_sb = singles.tile([C2 + C8, C], BF16, tag="wth")     # rows 64:80 used
        nc.sync.dma_start(out=wth_sb[C2 : C2 + C8, :], in_=wth_d)
        wog_sb = singles.tile([C2, C], BF16, tag="wog")          # [64, 128]
        nc.sync.dma_start(out=wog_sb, in_=wog_d)

        x_bf = [
            singles.tile([C, XT[t]], BF16, tag=f"xb{t}", name=f"{pfx}xb{t}")
            for t in range(len(XT))
        ]
        pool = singles.tile([C2 + C8, 32, 32], BF16, tag="pool")  # [80, 32, 32]
        pool_f = pool.rearrange("p a b -> p (a b)")
        a_sb = singles.tile([C, HWP], BF16, tag="a")             # A = w_theta^T @ phi
        gaug_all = singles.tile([C, 8, C], BF16, tag="gaug")
        e_sb = [
            singles.tile([C, HW], BF16, tag=f"e{j}", name=f"{pfx}e{j}")
            for j in range(8)
        ]
        r_all = singles.tile([C, 32], F32, tag="rall")
        stage = [
            singles.tile([C, 8, C], BF16, tag=f"stg{s}", name=f"{pfx}stg{s}")
            for s in range(2)
        ]
        onrm = singles.tile([C, HW], BF16, tag="onrm")  # rows 64:128 are junk
        t_sb = [
            singles.tile([C, 512], F32, tag=f"t{s}", name=f"{pfx}t{s}")
            for s in range(4)
        ]

        nc.gpsimd.memset(gaug_all[:, :, C2], 1.0)
        for s in range(2):
            # pad halves of the transpose stage: written once, never re-dirtied
            nc.gpsimd.memset(stage[s][:, :, C2:C], 0.0)

        # PSUM budget: sc 3x2 banks + ot 2 banks = 8. The pre-stage conv
        # chunks and the output-conv tiles rotate through the same sc pool
        # (no separate pools, so no pool-exit barriers), and the A-matrix
        # accumulates inside the ot tile (its copies drain before the first
        # o-accumulation overwrites the slots).
        with tc.tile_pool(name=pfx + "scA_psum", bufs=2, space="PSUM") as scA, \
             tc.tile_pool(name=pfx + "scD_psum", bufs=1, space="PSUM") as scD, \
             tc.tile_pool(name=pfx + "ot_psum", bufs=1, space="PSUM") as ot_psum:
            ps_o = ot_psum.tile([C, 8, 128], F32, tag="ot")
            a_ps = ps_o.rearrange("p k c -> p (k c)")  # [128, 1024] f32 view

            # ---- pre stage: x load -> conv -> 2x2 maxpool -> A chunks ------
            gp_tiles = {}
            for t in range(len(XT)):
                nc.gpsimd.dma_start(out=x_bf[t], in_=x_d[:, XO[t] : XO[t] + XT[t]])
                g0 = XO[t] // 1024  # which 1024-col gp allocation
                if g0 not in gp_tiles:
                    gpool = scA if g0 == 0 else scD
                    gp_tiles[g0] = gpool.tile(
                        [C, 1024], F32, tag="sc" if g0 == 0 else "scd",
                        name=f"{pfx}gp{g0}",
                    )
                w = XT[t]
                off = XO[t] % 1024
                ps_gp = gp_tiles[g0][0 : C2 + C8, off : off + w]
                step = min(512, w)
                for c0 in range(0, w, step):
                    nc.tensor.matmul(
                        ps_gp[:, c0 : c0 + step],
                        wgp_sb,
                        x_bf[t][:, c0 : c0 + step],
                    )
                # fused 2x2 maxpool: [80, (h, 2hp, 32w, 2wp)] -> [80, h, 32]
                nh = w // 128
                v = ps_gp.rearrange(
                    "p (h hp w wp) -> p h w hp wp", h=nh, hp=2, w=32, wp=2
                )
                pb = XO[t] // 128
                nc.vector.tensor_reduce(
                    out=pool[:, pb : pb + nh, :],
                    in_=v,
                    axis=mybir.AxisListType.XY,
                    op=mybir.AluOpType.max,
                )
                # A chunk = w_theta^T @ phi cols (K=16)
                pc = w // 4  # pooled cols of this chunk
                po = XO[t] // 4
                nc.tensor.matmul(
                    a_ps[:, po : po + pc],
                    wth_sb[C2 : C2 + C8, :],
                    pool_f[C2 : C2 + C8, po : po + pc],
                )
                nc.vector.tensor_copy(
                    out=a_sb[:, po : po + pc], in_=a_ps[:, po : po + pc]
                )
                # gaug blocks for the jc's fully covered by now
                for j in range(po // 128, (po + pc) // 128):
                    nc.sync.dma_start(
                        out=gaug_all[:, j, 0:C2],
                        in_=pool_f[0:C2, j * 128 : (j + 1) * 128],
                        transpose=True,
                    )

            # ---- main: scores -> exp -> oT accum -> norm -> conv -----------
            for icg in range(4):
                cbase = icg * 1024
                # scores^T tiles [128 j, 1024 i] + exp
                for jc in range(8):
                    dve = (icg, jc) in DVE_EXP
                    ps_sc = (scD if dve else scA).tile(
                        [C, 1024], F32, tag="scd" if dve else "sc",
                        name=f"{pfx}sc{icg}_{jc}",
                    )
                    col = 0
                    for t in range(len(XT)):
                        lo, hi = XO[t], XO[t] + XT[t]
                        s0, s1 = max(lo, cbase), min(hi, cbase + 1024)
                        if s0 >= s1:
                            continue
                        c0 = s0
                        while c0 < s1:
                            cw = min(512, s1 - c0)
                            nc.tensor.matmul(
                                ps_sc[:, col : col + cw],
                                a_sb[:, jc * 128 : (jc + 1) * 128],
                                x_bf[t][:, c0 - lo : c0 - lo + cw],
                            )
                            col += cw
                            c0 += cw
                    dst = e_sb[jc][:, cbase : cbase + 1024]
                    if dve:
                        nc.vector.tensor_scalar(
                            out=dst.bitcast(I16),
                            in0=ps_sc,
                            scalar1=SCH_A,
                            scalar2=SCH_B,
                            op0=mybir.AluOpType.mult,
                            op1=mybir.AluOpType.add,
                        )
                    else:
                        nc.scalar.activation(
                            out=dst, in_=ps_sc,
                            func=mybir.ActivationFunctionType.Exp,
                        )

                # oT accumulation: 8 i-chunks over 4 rotating psum slots
                st = stage[icg % 2]
                tail = icg == 3
                for kk in range(8):
                    cc = cbase + kk * 128
                    for jc in range(8):
                        nc.tensor.matmul(
                            ps_o[:, kk, 0 : C2 + 1],
                            e_sb[jc][:, cc : cc + 128],
                            gaug_all[:, jc, 0 : C2 + 1],
                            start=(jc == 0),
                            stop=(jc == 7),
                        )
                    if tail and kk % 2 == 1:
                        # chase norms pair-by-pair in the tail round
                        q = icg * 8 + kk - 1
                        nc.vector.reciprocal(
                            out=r_all[:, q : q + 2], in_=ps_o[:, kk - 1 : kk + 1, C2]
                        )
                        for kk2 in (kk - 1, kk):
                            nc.vector.tensor_scalar(
                                out=st[:, kk2, 0:C2],
                                in0=ps_o[:, kk2, 0:C2],
                                scalar1=r_all[:, icg * 8 + kk2 : icg * 8 + kk2 + 1],
                                scalar2=None,
                                op0=mybir.AluOpType.mult,
                            )
                if not tail:
                    nc.vector.reciprocal(
                        out=r_all[:, icg * 8 : icg * 8 + 8], in_=ps_o[:, :, C2]
                    )
                    for kk in range(8):
                        nc.vector.tensor_scalar(
                            out=st[:, kk, 0:C2],
                            in0=ps_o[:, kk, 0:C2],
                            scalar1=r_all[:, icg * 8 + kk : icg * 8 + kk + 1],
                            scalar2=None,
                            op0=mybir.AluOpType.mult,
                        )
                ps_cv = scD.tile([C, 1024], F32, tag="scd", name=f"{pfx}cv{icg}")
                # transposes: halves normally, quarters in the tail round
                nt = 4 if tail else 2
                kper = 8 // nt
                for b in range(nt):
                    dstt = onrm[
                        :, cbase + b * kper * 128 : cbase + (b + 1) * kper * 128
                    ].rearrange("c (k i) -> c k i", k=kper, i=128)
                    nc.sync.dma_start(
                        out=dstt,
                        in_=st[:, b * kper : (b + 1) * kper, :],
                        transpose=True,
                    )
                # output conv + residual + store per 512-col chunk
                for b in range(2):
                    h = 2 * icg + b
                    nsp = 2 if tail else 1
                    ps_oc = ps_cv[:, b * 512 : (b + 1) * 512]
                    for c0 in range(0, 512, 512 // nsp):
                        cw = 512 // nsp
                        nc.tensor.matmul(
                            ps_oc[:, c0 : c0 + cw],
                            wog_sb,
                            onrm[0:C2, h * 512 + c0 : h * 512 + c0 + cw],
                        )
                    t = t_sb[h % 4]
                    for tt in range(len(XT)):
                        lo, hi = XO[tt], XO[tt] + XT[tt]
                        s0, s1 = max(lo, h * 512), min(hi, h * 512 + 512)
                        if s0 >= s1:
                            continue
                        nc.vector.tensor_add(
                            t[:, s0 - h * 512 : s1 - h * 512],
                            ps_oc[:, s0 - h * 512 : s1 - h * 512],
                            x_bf[tt][:, s0 - lo : s1 - lo],
                        )
                    for c0 in range(0, 512, 512 // nsp):
                        cw = 512 // nsp
                        out_eng = nc.sync if (h + c0 // 256) % 2 == 0 else nc.gpsimd
                        out_eng.dma_start(
                            out=out_d[:, h * 512 + c0 : h * 512 + c0 + cw],
                            in_=t[:, c0 : c0 + cw],
                        )


def _build(nreps=1):
    nc = bacc.Bacc(None)
    x_d = nc.declare_dram_parameter("x", [C, HW], F32, isOutput=False)
    wgp_d = nc.declare_dram_parameter("w_gpT", [C, C2 + C8], BF16, isOutput=False)
    wth_d = nc.declare_dram_parameter("w_th", [C8, C], BF16, isOutput=False)
    wog_d = nc.declare_dram_parameter("w_og", [C2, C], BF16, isOutput=False)
    out_d = nc.declare_dram_parameter("out", [C, HW], F32, isOutput=True)
    with tile.TileContext(nc) as tc:
        for rep in range(nreps):
            _emit(nc, tc, x_d.ap(), wgp_d.ap(), wth_d.ap(), wog_d.ap(), out_d.ap(),
                  pfx=f"r{rep}_" if nreps > 1 else "")
    nc.compile()
    return nc


_NC = None


def _get_nc():
    global _NC
    if _NC is None:
        _NC = _build()
    return _NC


def _host_weights(w_theta, w_phi, w_g, w_o, gamma):
    import ml_dtypes

    w_theta = np.asarray(w_theta, np.float32)
    w_phi = np.asarray(w_phi, np.float32)
    w_g = np.asarray(w_g, np.float32)
    w_o = np.asarray(w_o, np.float32)
    gamma = np.float32(np.asarray(gamma))
    # stationary [128, 80]: columns 0:64 -> g rows, 64:80 -> phi rows
    w_gpT = np.ascontiguousarray(np.concatenate([w_g, w_phi], 0).T).astype(
        ml_dtypes.bfloat16
    )
    w_th = np.ascontiguousarray(w_theta).astype(ml_dtypes.bfloat16)
    # [64, 128] = (gamma*w_o)^T
    w_og = np.ascontiguousarray((gamma * w_o).T).astype(ml_dtypes.bfloat16)
    return w_gpT, w_th, w_og


def kernel(inputs, w_theta, w_phi, w_g, w_o, gamma):
    global LAST_RESULTS
    x = np.ascontiguousarray(np.asarray(inputs, np.float32)).reshape(NCORES, C, HW)
    w_gpT, w_th, w_og = _host_weights(w_theta, w_phi, w_g, w_o, gamma)
    nc = _get_nc()
    in_maps = [
        {"x": x[b], "w_gpT": w_gpT, "w_th": w_th, "w_og": w_og}
        for b in range(NCORES)
    ]
    res = run_bass_kernel_spmd(nc, in_maps, list(range(NCORES)))
    LAST_RESULTS = res
    out = np.stack([res.results[b]["out"] for b in range(NCORES)])
    return out.reshape(NCORES, C, 64, 64).astype(np.float32, copy=False)


# revision 26
# speedup vs baseline: 1.0699x; 1.0699x over previous
"""SAGAN-style self-attention block on 8 Trainium2 NeuronCores.

Reference computation (per batch element b, data-parallel over B=8):
    theta = w_theta @ x                         [16, 4096]
    phi   = maxpool2x2(w_phi @ x)               [16, 1024]
    g     = maxpool2x2(w_g @ x)                 [64, 1024]
    scores= theta^T @ phi                       [4096, 1024]
    beta  = softmax_j(scores)
    o     = g @ beta^T                          [64, 4096]
    out   = gamma * (w_o @ o) + x               [128, 4096]

Device algorithm (one batch element per core):
  * theta is never materialized: scores^T = A^T @ x with A = w_theta^T @ phi,
    so the big matmul runs with K=128 instead of K=16.
  * scores^T is kept j-on-partitions; softmax runs without max subtraction
    (max |score| ~ 37 for this distribution, exp stays in fp32/bf16 range).
  * exp is split across the Activation engine (exact) and the Vector engine
    (Schraudolph bit-trick exp via tensor_scalar f32->int16 bitcast bf16,
    ~3% per-element error, calibrated for the HW's round-to-nearest).
  * o is accumulated TRANSPOSED: out[i, c] = sum_j E[j, i] * gaug[j, c] with
    i on all 128 partitions (PE cost is per output column, so [128, 65] out
    beats [65, 512] out by ~2x). gaug's column 64 is all-ones, which makes
    the softmax denominator land as a per-partition column for free.
  * normalization = reciprocal of the denominator column [128, 4] + a
    per-partition-scalar tensor_scalar multiply fused into the mandatory
    PSUM->SBUF evacuation (bf16 out).
  * o_norm^T tiles are transposed back to [c, i] with one batched XBAR
    transpose DMA per icg round (src [128, (64c x 8k)], 3D dest AP).
  * output conv (gamma folded into w_o on host) + residual add (reads the
    bf16 x copy) evacuates the conv PSUM via one DVE tensor_add per chunk.
"""

import numpy as np

import concourse.bass as bass
import concourse.bacc as bacc
import concourse.tile as tile
from concourse import mybir
from concourse.bass_utils import run_bass_kernel_spmd

F32 = mybir.dt.float32
BF16 = mybir.dt.bfloat16
I16 = mybir.dt.int16

C = 128          # channels
HW = 4096        # 64*64 spatial
HWP = 1024       # pooled spatial (32*32)
C8 = 16          # C // 8
C2 = 64          # C // 2
NCORES = 8

# Schraudolph fast-exp constants (bf16 via int16): E = bitcast(int16(A*s + B)).
# Bias calibrated for round-to-nearest (what HW does; sim truncates, which
# only matters for sim-side numerics, not timing).
SCH_A = float(2**7 / np.log(2.0))
SCH_B = float(127 * 2**7) - 5.5

# x tile layout: sizes (in cols) of the bf16 x tiles; first two smaller so
# the pre-phase pipeline starts earlier.
XT = (256, 256, 512, 512, 512, 1024, 1024)
XO = (0, 256, 512, 1024, 1536, 2048, 3072)

# (icg, jc) pairs whose exp runs on the Vector engine via fast-exp.
DVE_EXP = {(0, 4), (0, 5), (1, 3), (1, 6), (2, 3), (2, 6), (3, 2)}

LAST_RESULTS = None


def _emit(nc: bass.Bass, tc: tile.TileContext, x_d, wgp_d, wth_d, wog_d, out_d, pfx=""):
    import contextlib

    with contextlib.ExitStack() as ctx:
        singles = ctx.enter_context(tc.tile_pool(name=pfx + "singles", bufs=1))

        # dummy exp so the ACT function table loads at kernel start
        dummy = singles.tile([1, 1], F32, tag="dummy")
        nc.vector.memset(dummy, 0.0)
        nc.scalar.activation(out=dummy, in_=dummy, func=mybir.ActivationFunctionType.Exp)

        # ---- constants / weights (triggered from ACT: idle during prologue) --
        wgp_sb = singles.tile([C, C2 + C8], BF16, tag="wgp")     # [128, 80]
        nc.sync.dma_start(out=wgp_sb, in_=wgp_d)
        wth_sb = singles.tile([C2 + C8, C], BF16, tag="wth")     # rows 64:80 used
        nc.sync.dma_start(out=wth_sb[C2 : C2 + C8, :], in_=wth_d)
        wog_sb = singles.tile([C2, C], BF16, tag="wog")          # [64, 128]
        nc.sync.dma_start(out=wog_sb, in_=wog_d)

        x_bf = [
            singles.tile([C, XT[t]], BF16, tag=f"xb{t}", name=f"{pfx}xb{t}")
            for t in range(len(XT))
        ]
        pool = singles.tile([C2 + C8, 32, 32], BF16, tag="pool")  # [80, 32, 32]
        pool_f = pool.rearrange("p a b -> p (a b)")
        a_sb = singles.tile([C, HWP], BF16, tag="a")             # A = w_theta^T @ phi
        gaug_all = singles.tile([C, 8, C], BF16, tag="gaug")
        e_sb = [
            singles.tile([C, HW], BF16, tag=f"e{j}", name=f"{pfx}e{j}")
            for j in range(8)
        ]
        r_all = singles.tile([C, 32], F32, tag="rall")
        stage = [
            singles.tile([C, 8, C], BF16, tag=f"stg{s}", name=f"{pfx}stg{s}")
            for s in range(2)
        ]
        onrm = singles.tile([C, HW], BF16, tag="onrm")  # rows 64:128 are junk
        t_sb = [
            singles.tile([C, 512], F32, tag=f"t{s}", name=f"{pfx}t{s}")
            for s in range(4)
        ]

        nc.gpsimd.memset(gaug_all[:, :, C2], 1.0)
        for s in range(2):
            # pad halves of the transpose stage: written once, never re-dirtied
            nc.gpsimd.memset(stage[s][:, :, C2:C], 0.0)

        # PSUM budget: sc 3x2 banks + ot 2 banks = 8. The pre-stage conv
        # chunks and the output-conv tiles rotate through the same sc pool
        # (no separate pools, so no pool-exit barriers), and the A-matrix
        # accumulates inside the ot tile (its copies drain before the first
        # o-accumulation overwrites the slots).
        with tc.tile_pool(name=pfx + "scA_psum", bufs=2, space="PSUM") as scA, \
             tc.tile_pool(name=pfx + "scD_psum", bufs=1, space="PSUM") as scD, \
             tc.tile_pool(name=pfx + "ot_psum", bufs=1, space="PSUM") as ot_psum:
            ps_o = ot_psum.tile([C, 8, 128], F32, tag="ot")
            a_ps = ps_o.rearrange("p k c -> p (k c)")  # [128, 1024] f32 view

            def emit_scores_exp(icg, jc):
                cbase = icg * 1024
                dve = (icg, jc) in DVE_EXP
                ps_sc = (scD if dve else scA).tile(
                    [C, 1024], F32, tag="scd" if dve else "sc",
                    name=f"{pfx}sc{icg}_{jc}",
                )
                col = 0
                for t in range(len(XT)):
                    lo, hi = XO[t], XO[t] + XT[t]
                    s0, s1 = max(lo, cbase), min(hi, cbase + 1024)
                    if s0 >= s1:
                        continue
                    c0 = s0
                    while c0 < s1:
                        cw = min(512, s1 - c0)
                        nc.tensor.matmul(
                            ps_sc[:, col : col + cw],
                            a_sb[:, jc * 128 : (jc + 1) * 128],
                            x_bf[t][:, c0 - lo : c0 - lo + cw],
                        )
                        col += cw
                        c0 += cw
                dst = e_sb[jc][:, cbase : cbase + 1024]
                if dve:
                    nc.vector.tensor_scalar(
                        out=dst.bitcast(I16),
                        in0=ps_sc,
                        scalar1=SCH_A,
                        scalar2=SCH_B,
                        op0=mybir.AluOpType.mult,
                        op1=mybir.AluOpType.add,
                    )
                else:
                    nc.scalar.activation(
                        out=dst, in_=ps_sc,
                        func=mybir.ActivationFunctionType.Exp,
                    )

            # ---- pre stage: x load -> conv -> 2x2 maxpool -> A chunks ------
            gp_tiles = {}
            for t in range(len(XT)):
                nc.gpsimd.dma_start(out=x_bf[t], in_=x_d[:, XO[t] : XO[t] + XT[t]])
                g0 = XO[t] // 1024  # which 1024-col gp allocation
                if g0 not in gp_tiles:
                    gpool = scA if g0 == 0 else scD
                    gp_tiles[g0] = gpool.tile(
                        [C, 1024], F32, tag="sc" if g0 == 0 else "scd",
                        name=f"{pfx}gp{g0}",
                    )
                w = XT[t]
                off = XO[t] % 1024
                ps_gp = gp_tiles[g0][0 : C2 + C8, off : off + w]
                step = min(512, w)
                for c0 in range(0, w, step):
                    nc.tensor.matmul(
                        ps_gp[:, c0 : c0 + step],
                        wgp_sb,
                        x_bf[t][:, c0 : c0 + step],
                    )
                # fused 2x2 maxpool: [80, (h, 2hp, 32w, 2wp)] -> [80, h, 32]
                nh = w // 128
                v = ps_gp.rearrange(
                    "p (h hp w wp) -> p h w hp wp", h=nh, hp=2, w=32, wp=2
                )
                pb = XO[t] // 128
                nc.vector.tensor_reduce(
                    out=pool[:, pb : pb + nh, :],
                    in_=v,
                    axis=mybir.AxisListType.XY,
                    op=mybir.AluOpType.max,
                )
                # A chunk = w_theta^T @ phi cols (K=16)
                pc = w // 4  # pooled cols of this chunk
                po = XO[t] // 4
                nc.tensor.matmul(
                    a_ps[:, po : po + pc],
                    wth_sb[C2 : C2 + C8, :],
                    pool_f[C2 : C2 + C8, po : po + pc],
                )
                nc.vector.tensor_copy(
                    out=a_sb[:, po : po + pc], in_=a_ps[:, po : po + pc]
                )
                # gaug blocks for the jc's fully covered by now
                for j in range(po // 128, (po + pc) // 128):
                    nc.sync.dma_start(
                        out=gaug_all[:, j, 0:C2],
                        in_=pool_f[0:C2, j * 128 : (j + 1) * 128],
                        transpose=True,
                    )
                # round-0 scores/exp for jc's whose A-block just completed
                # (emitting here keeps the in-order PE queue from stalling
                # round 0 behind the last x tile)
                if XO[t] + XT[t] >= 1024:
                    jhi = (XO[t] + XT[t]) // 512 - 1
                    jlo = max(XO[t] // 512 - 1, 0) if XO[t] >= 1024 else 0
                    for jc in range(jlo, jhi):
                        emit_scores_exp(0, jc)

            # ---- main: scores -> exp -> oT accum -> norm -> conv -----------
            for icg in range(4):
                cbase = icg * 1024
                # scores^T tiles [128 j, 1024 i] + exp
                if icg == 0:
                    for jc in (6, 7):
                        emit_scores_exp(0, jc)
                else:
                    for jc in range(8):
                        emit_scores_exp(icg, jc)

                # oT accumulation: 8 i-chunks over 4 rotating psum slots
                st = stage[icg % 2]
                tail = icg == 3
                for kk in range(8):
                    cc = cbase + kk * 128
                    for jc in range(8):
                        nc.tensor.matmul(
                            ps_o[:, kk, 0 : C2 + 1],
                            e_sb[jc][:, cc : cc + 128],
                            gaug_all[:, jc, 0 : C2 + 1],
                            start=(jc == 0),
                            stop=(jc == 7),
                        )
                    if tail and kk % 2 == 1:
                        # chase norms pair-by-pair in the tail round
                        q = icg * 8 + kk - 1
                        nc.vector.reciprocal(
                            out=r_all[:, q : q + 2], in_=ps_o[:, kk - 1 : kk + 1, C2]
                        )
                        for kk2 in (kk - 1, kk):
                            nc.vector.tensor_scalar(
                                out=st[:, kk2, 0:C2],
                                in0=ps_o[:, kk2, 0:C2],
                                scalar1=r_all[:, icg * 8 + kk2 : icg * 8 + kk2 + 1],
                                scalar2=None,
                                op0=mybir.AluOpType.mult,
                            )
                if not tail:
                    nc.vector.reciprocal(
                        out=r_all[:, icg * 8 : icg * 8 + 8], in_=ps_o[:, :, C2]
                    )
                    for kk in range(8):
                        nc.vector.tensor_scalar(
                            out=st[:, kk, 0:C2],
                            in0=ps_o[:, kk, 0:C2],
                            scalar1=r_all[:, icg * 8 + kk : icg * 8 + kk + 1],
                            scalar2=None,
                            op0=mybir.AluOpType.mult,
                        )
                if tail:
                    ps_cv = scD.tile([C, 1024], F32, tag="scd", name=f"{pfx}cv{icg}")
                # transposes: halves normally, quarters in the tail round
                nt = 4 if tail else 2
                kper = 8 // nt
                for b in range(nt):
                    dstt = onrm[
                        :, cbase + b * kper * 128 : cbase + (b + 1) * kper * 128
                    ].rearrange("c (k i) -> c k i", k=kper, i=128)
                    nc.sync.dma_start(
                        out=dstt,
                        in_=st[:, b * kper : (b + 1) * kper, :],
                        transpose=True,
                    )
                # output conv + residual + store per 512-col chunk
                for b in range(2):
                    h = 2 * icg + b
                    nsp = 2 if tail else 1
                    if tail:
                        ps_oc = ps_cv[:, b * 512 : (b + 1) * 512]
                    else:
                        ps_oc = ps_o[:, 4 * b : 4 * b + 4, :].rearrange(
                            "p k c -> p (k c)"
                        )
                    for c0 in range(0, 512, 512 // nsp):
                        cw = 512 // nsp
                        nc.tensor.matmul(
                            ps_oc[:, c0 : c0 + cw],
                            wog_sb,
                            onrm[0:C2, h * 512 + c0 : h * 512 + c0 + cw],
                        )
                    t = t_sb[h % 4]
                    for tt in range(len(XT)):
                        lo, hi = XO[tt], XO[tt] + XT[tt]
                        s0, s1 = max(lo, h * 512), min(hi, h * 512 + 512)
                        if s0 >= s1:
                            continue
                        nc.vector.tensor_add(
                            t[:, s0 - h * 512 : s1 - h * 512],
                            ps_oc[:, s0 - h * 512 : s1 - h * 512],
                            x_bf[tt][:, s0 - lo : s1 - lo],
                        )
                    for c0 in range(0, 512, 512 // nsp):
                        cw = 512 // nsp
                        out_eng = nc.sync if (h + c0 // 256) % 2 == 0 else nc.gpsimd
                        out_eng.dma_start(
                            out=out_d[:, h * 512 + c0 : h * 512 + c0 + cw],
                            in_=t[:, c0 : c0 + cw],
                        )


def _build(nreps=1):
    nc = bacc.Bacc(None)
    x_d = nc.declare_dram_parameter("x", [C, HW], F32, isOutput=False)
    wgp_d = nc.declare_dram_parameter("w_gpT", [C, C2 + C8], BF16, isOutput=False)
    wth_d = nc.declare_dram_parameter("w_th", [C8, C], BF16, isOutput=False)
    wog_d = nc.declare_dram_parameter("w_og", [C2, C], BF16, isOutput=False)
    out_d = nc.declare_dram_parameter("out", [C, HW], F32, isOutput=True)
    with tile.TileContext(nc) as tc:
        for rep in range(nreps):
            _emit(nc, tc, x_d.ap(), wgp_d.ap(), wth_d.ap(), wog_d.ap(), out_d.ap(),
                  pfx=f"r{rep}_" if nreps > 1 else "")
    nc.compile()
    return nc


_NC = None


def _get_nc():
    global _NC
    if _NC is None:
        _NC = _build()
    return _NC


def _host_weights(w_theta, w_phi, w_g, w_o, gamma):
    import ml_dtypes

    w_theta = np.asarray(w_theta, np.float32)
    w_phi = np.asarray(w_phi, np.float32)
    w_g = np.asarray(w_g, np.float32)
    w_o = np.asarray(w_o, np.float32)
    gamma = np.float32(np.asarray(gamma))
    # stationary [128, 80]: columns 0:64 -> g rows, 64:80 -> phi rows
    w_gpT = np.ascontiguousarray(np.concatenate([w_g, w_phi], 0).T).astype(
        ml_dtypes.bfloat16
    )
    w_th = np.ascontiguousarray(w_theta).astype(ml_dtypes.bfloat16)
    # [64, 128] = (gamma*w_o)^T
    w_og = np.ascontiguousarray((gamma * w_o).T).astype(ml_dtypes.bfloat16)
    return w_gpT, w_th, w_og


def kernel(inputs, w_theta, w_phi, w_g, w_o, gamma):
    global LAST_RESULTS
    x = np.ascontiguousarray(np.asarray(inputs, np.float32)).reshape(NCORES, C, HW)
    w_gpT, w_th, w_og = _host_weights(w_theta, w_phi, w_g, w_o, gamma)
    nc = _get_nc()
    in_maps = [
        {"x": x[b], "w_gpT": w_gpT, "w_th": w_th, "w_og": w_og}
        for b in range(NCORES)
    ]
    res = run_bass_kernel_spmd(nc, in_maps, list(range(NCORES)))
    LAST_RESULTS = res
    out = np.stack([res.results[b]["out"] for b in range(NCORES)])
    return out.reshape(NCORES, C, 64, 64).astype(np.float32, copy=False)
